# revision 1
# baseline (speedup 1.0000x reference)
"""Deformable alignment fusion kernel for TRN2, 8-core data-parallel.

Math (per batch b):
  cat    = concat([low, high], ch)                       # (256, H, W)
  offset = conv3x3(cat, w_off) + b_off                   # (18, H, W)  (dy,dx)*9 taps
  aligned= deform_conv(low, offset, w_def) + b_def       # (128, H, W)
  gate   = sigmoid(w_mod @ cat + b_mod)                  # (128, H, W)
  out    = aligned * gate + high

Sharding: core i handles batch b = i//2, rows [64*(i%2), 64*(i%2)+64).

Device algorithm per core (channel-major, fp16 matmul operands):
 - offset conv: direct 3x3 conv as 18 accumulating matmuls per 512-px chunk.
 - bilinear sampling in "monomial" form: S = P0 + wx*P1 + wy*P2 + wx*wy*P3
   where P0..P3 are the value / x-diff / y-diff / xy-diff planes of the
   (guard-padded) low image, all gathered at the single flat index
   i0 = floor(py)*136 + floor(px) from a host-prepared pixel-major
   4-plane table via dma_gather(transpose=True)  -> channel-major tiles.
 - the deform conv contraction folds the monomial sum into PSUM:
   psum += w_def_k.T @ P0 ; += w_def_k.T @ (wx*P1) ; ...  (4 matmuls/tap)
 - per-pixel weights wx, wy are rows broadcast to 128 partitions with
   gpsimd.partition_broadcast; wx*wy computed on DVE from the broadcasts.
 - gate: two 1x1 matmuls + Sigmoid on the scalar engine.
"""

import numpy as np

import concourse.bass as bass
import concourse.tile as tile
from concourse import bacc, mybir
from concourse.bass import ts

dt = mybir.dt
F16 = dt.float16
F32 = dt.float32
I16 = dt.int16
Alu = mybir.AluOpType
Act = mybir.ActivationFunctionType

B, C, H, W = 4, 128, 128, 128
GP = 4                 # guard pad for sampling
HP = H + 2 * GP        # 136
NP = HP * HP           # 18496 padded pixels
HR = 64                # rows per core
NPIX = HR * W          # 8192 pixels per core
NQ = 4                 # quarters per core
QP = NPIX // NQ        # 2048 pixels per quarter
QR = QP // W           # 16 rows per quarter
CLAMP_HI = float(H + 2 * GP - 2)  # 134.0 : floor+1 stays inside padded image

# offset channel regrouping: rows 0..8 = dy(tap), rows 9..17 = dx(tap)
PERM = [2 * k for k in range(9)] + [2 * k + 1 for k in range(9)]


def _ap(t, offset, dims):
    """Raw AP on the same tensor as AP `t`, with explicit [step, count] dims."""
    return bass.AP(tensor=t.tensor, offset=t.offset + offset, ap=list(dims))


DEBUG_DUMPS = False


def build_program():
    nc = bacc.Bacc("TRN2", debug=False)

    io = {}

    def din(name, shape, d):
        io[name] = nc.dram_tensor(name, shape, d, kind="ExternalInput").ap()
        return io[name]

    din("src4", [NP, 512], F16)           # [pix, (4 planes x 128 ch)]
    din("lowp", [128, 66 * 130], F16)     # rows h0-1..h1+1, W-padded by 1
    din("highp", [128, 66 * 130], F16)
    din("highc", [128, NPIX], F32)        # center high rows, f32
    din("w_off_t", [2, 3, 3, 128, 18], F16)
    din("w_def_t", [9, 128, 128], F16)
    din("w_mod_t", [2, 128, 128], F16)
    din("b_off_g", [18, 1], F32)
    din("b_def_c", [128, 1], F32)
    din("b_mod_c", [128, 1], F32)
    din("base_w", [18, NPIX], F16)        # sampling-position bases, wrapped order
    io["idx_scr"] = nc.dram_tensor("idx_scr", [9, NPIX], I16, kind="Internal").ap()
    if DEBUG_DUMPS:
        io["dbg_pos"] = nc.dram_tensor("dbg_pos", [18, NPIX], F32, kind="ExternalOutput").ap()
        io["dbg_gate"] = nc.dram_tensor("dbg_gate", [128, NPIX], F16, kind="ExternalOutput").ap()
        io["dbg_fracx"] = nc.dram_tensor("dbg_fracx", [9, NPIX], F16, kind="ExternalOutput").ap()
        io["dbg_fracy"] = nc.dram_tensor("dbg_fracy", [9, NPIX], F16, kind="ExternalOutput").ap()
        io["dbg_idx"] = nc.dram_tensor("dbg_idx", [9, NPIX], I16, kind="ExternalOutput").ap()
    out_d = nc.dram_tensor("out", [128, NPIX], F32, kind="ExternalOutput").ap()

    with tile.TileContext(nc) as tc:
        trace_kernel(tc, io, out_d)

    nc.compile()
    return nc


def trace_kernel(tc, io, out_d):
    nc = tc.nc
    from contextlib import ExitStack

    ctx = ExitStack()
    consts = ctx.enter_context(tc.tile_pool(name="consts", bufs=1))
    npool = ctx.enter_context(tc.tile_pool(name="narrow", bufs=1))
    spool = ctx.enter_context(tc.tile_pool(name="small", bufs=2))
    s1pool = ctx.enter_context(tc.tile_pool(name="small1", bufs=1))
    ppool = ctx.enter_context(tc.tile_pool(name="ps_small", bufs=2, space="PSUM"))
    dpool = ctx.enter_context(tc.tile_pool(name="ps_deform", bufs=1, space="PSUM"))

    # ---------------- constants to SBUF ----------------
    w_off_sb = consts.tile([128, 2, 3, 3, 18], F16)
    nc.sync.dma_start(
        w_off_sb[:], io["w_off_t"].rearrange("cb ky kx c o -> c cb ky kx o")
    )
    w_def_sb = consts.tile([128, 9, 128], F16)
    nc.sync.dma_start(w_def_sb[:], io["w_def_t"].rearrange("k c o -> c k o"))
    w_mod_sb = consts.tile([128, 2, 128], F16)
    nc.sync.dma_start(w_mod_sb[:], io["w_mod_t"].rearrange("cb c o -> c cb o"))
    b_off_sb = consts.tile([18, 1], F32)
    nc.sync.dma_start(b_off_sb[:], io["b_off_g"])
    b_def_sb = consts.tile([128, 1], F32)
    nc.sync.dma_start(b_def_sb[:], io["b_def_c"])
    b_mod_sb = consts.tile([128, 1], F32)
    nc.sync.dma_start(b_mod_sb[:], io["b_mod_c"])
    gate_sb = npool.tile([128, NPIX], F16, tag="gate")

    # ---------------- stage A: offset conv -> sampling positions ----------------
    # pos_sb holds, in "wrapped" column order t = p16*128 + s (per quarter),
    # rows 0..8:  py + GP  for taps 0..8     rows 9..17: px + GP
    stageAB = ExitStack()
    imgpool = stageAB.enter_context(tc.tile_pool(name="imgs", bufs=1))
    bigpool = stageAB.enter_context(tc.tile_pool(name="bigtmp", bufs=1))
    base_sb = imgpool.tile([18, NPIX], F16)
    nc.sync.dma_start(base_sb[:], io["base_w"])
    pos_sb = bigpool.tile([18, NPIX], F32, tag="pos")
    if True:
        lowp_sb = imgpool.tile([128, 66, 130], F16)
        nc.sync.dma_start(lowp_sb[:], io["lowp"].rearrange("c (h w) -> c h w", h=66))
        highp_sb = imgpool.tile([128, 66, 130], F16)
        nc.sync.dma_start(highp_sb[:], io["highp"].rearrange("c (h w) -> c h w", h=66))

        for q in range(NQ):
            for cc in range(4):  # 4 chunks of 512 px (4 image rows) per quarter
                ps = ppool.tile([18, 512], F32, tag="offps")
                r0 = q * QR + cc * 4
                n_mm = 0
                for cb in range(2):
                    pad = lowp_sb if cb == 0 else highp_sb
                    for ky in range(3):
                        for kx in range(3):
                            nc.tensor.matmul(
                                ps[:],
                                lhsT=w_off_sb[:, cb, ky, kx, :],
                                rhs=pad[:, r0 + ky : r0 + ky + 4, kx : kx + 128],
                                start=(n_mm == 0),
                                stop=(n_mm == 17),
                            )
                            n_mm += 1
                # evacuate with wrapped reorder: dest (p16, s) <- psum col s*16+p16
                ps3 = ps[:].rearrange("r (s p) -> r p s", p=16)  # [18, 16, 32]
                dest = pos_sb[:].rearrange("r (q p s) -> r q p s", q=NQ, p=16)[
                    :, q, :, cc * 32 : cc * 32 + 32
                ]
                base3 = base_sb[:].rearrange("r (q p s) -> r q p s", q=NQ, p=16)[
                    :, q, :, cc * 32 : cc * 32 + 32
                ]
                nc.vector.scalar_tensor_tensor(
                    out=dest, in0=ps3, scalar=b_off_sb[:], in1=base3,
                    op0=Alu.add, op1=Alu.add,
                )

        # ---------------- stage C: gate ----------------
        for ch in range(16):
            r0 = ch * 4
            psg = ppool.tile([128, 512], F32, tag="gateps")
            for cb in range(2):
                pad = lowp_sb if cb == 0 else highp_sb
                nc.tensor.matmul(
                    psg[:],
                    lhsT=w_mod_sb[:, cb, :],
                    rhs=pad[:, 1 + r0 : 1 + r0 + 4, 1:129],
                    start=(cb == 0),
                    stop=(cb == 1),
                )
            nc.scalar.activation(
                out=gate_sb[:, ts(ch, 512)], in_=psg[:],
                func=Act.Sigmoid, bias=b_mod_sb[:], scale=1.0,
            )

    if DEBUG_DUMPS:
        nc.sync.dma_start(io["dbg_pos"], pos_sb[:])
        nc.sync.dma_start(io["dbg_gate"], gate_sb[:])
    # ---------------- stage B: narrow index / weight math ----------------
    posx = bigpool.tile([9, NPIX], F32, tag="posx")
    nc.gpsimd.dma_start(posx[:], pos_sb[9:18, :])  # rows 9..17 -> partitions 0..8
    # clamp in place
    nc.vector.tensor_scalar(
        pos_sb[0:9, :], pos_sb[0:9, :], 0.0, CLAMP_HI, Alu.max, Alu.min
    )
    nc.vector.tensor_scalar(posx[:], posx[:], 0.0, CLAMP_HI, Alu.max, Alu.min)
    # frac = pos - floor(pos); floor(v) = ((v - 0.5) + 2^23) - 2^23 for v >= 0
    # (round-to-nearest-even of v-0.5; differs from floor only at odd-integer
    #  v, where frac=1.0 gives the same bilinear result by continuity)
    MAGIC_A = 8388608.0 - 0.5
    MAGIC_B = 8388608.0
    ftmp = npool.tile([9, NPIX], F16, tag="ftmp_idxr")
    nc.vector.tensor_scalar(ftmp[:], pos_sb[0:9, :], MAGIC_A, MAGIC_B, Alu.add, Alu.subtract)
    fracy = npool.tile([9, NPIX], F16, tag="fracy")
    nc.vector.tensor_tensor(fracy[:], pos_sb[0:9, :], ftmp[:], Alu.subtract)
    nc.vector.tensor_scalar(ftmp[:], posx[:], MAGIC_A, MAGIC_B, Alu.add, Alu.subtract)
    fracx = npool.tile([9, NPIX], F16, tag="fracx")
    nc.vector.tensor_tensor(fracx[:], posx[:], ftmp[:], Alu.subtract)
    # i0 = floor(py)*HP + floor(px) = (py*HP + px) - (wy*HP + wx), rounded
    nc.vector.scalar_tensor_tensor(
        out=pos_sb[0:9, :], in0=pos_sb[0:9, :], scalar=float(HP), in1=posx[:],
        op0=Alu.mult, op1=Alu.add,
    )
    nc.vector.scalar_tensor_tensor(
        out=posx[:], in0=fracy[:], scalar=float(HP), in1=fracx[:],
        op0=Alu.mult, op1=Alu.add,
    )
    idx16 = npool.tile([9, NPIX], I16, tag="idx16")
    # (af - bf) is an exact integer +- ~0.07 fp noise; the DVE output cast
    # rounds to nearest, which lands it exactly.
    nc.vector.scalar_tensor_tensor(
        out=idx16[:], in0=pos_sb[0:9, :], scalar=0.0, in1=posx[:],
        op0=Alu.add, op1=Alu.subtract,
    )
    if DEBUG_DUMPS:
        nc.sync.dma_start(io["dbg_fracx"], fracx[:])
        nc.sync.dma_start(io["dbg_fracy"], fracy[:])
        nc.sync.dma_start(io["dbg_idx"], idx16[:])
    stageAB.close()
    # round-trip through DRAM to replicate the wrapped idx rows to all
    # 8 Q7 core groups: idxr[p, k, q*128 + s] = idx16[k, q*2048 + (p%16)*128 + s]
    nc.sync.dma_start(io["idx_scr"], idx16[:])
    idxr = npool.tile([128, 9, 4, 128], I16, tag="ftmp_idxr")
    for k in range(9):
        for q in range(NQ):
            rep_ap = _ap(
                io["idx_scr"], k * NPIX + q * QP,
                [[0, 8], [128, 16], [1, 128]],
            )
            nc.sync.dma_start(idxr[:, k, q, :], rep_ap)

    # ---------------- stage D: deformable conv ----------------
    gpool = ctx.enter_context(tc.tile_pool(name="gather", bufs=2))
    wpool = ctx.enter_context(tc.tile_pool(name="wfield", bufs=4))
    tpool = ctx.enter_context(tc.tile_pool(name="tplanes", bufs=2))
    for q in range(NQ):
        dps = dpool.tile([128, QP], F32)  # 4 PSUM banks
        for k in range(9):
            # (2) gather the 4 planes at i0, transposed to channel-major
            G = gpool.tile([128, 4, QP], F16)
            nc.gpsimd.dma_gather(
                out_ap=G[:],
                in_ap=io["src4"],
                idxs_ap=idxr[:, k, q, :],
                num_idxs=QP,
                num_idxs_reg=QP,
                elem_size=512,
                transpose=True,
                single_packet=False,
            )
            # (3) broadcast the per-pixel weights, unwrapping to natural order
            wts = []
            for srcrow in (fracx, fracy):
                stg = s1pool.tile([1, QP], F16, tag="stg")
                nc.sync.dma_start(stg[:], srcrow[k : k + 1, ts(q, QP)])
                wt = wpool.tile([128, QP], F16, tag="w")
                iap = bass.AP(
                    tensor=stg[:].tensor, offset=stg[:].offset,
                    ap=[list(stg[:].ap[0]), [1, 128], [128, 16]],
                )
                nc.gpsimd.partition_broadcast(wt[:], iap, channels=128)
                wts.append(wt)
            wx_t, wy_t = wts
            # (4) weighted planes
            T = tpool.tile([128, 3, QP], F16, tag="t")
            nc.vector.tensor_tensor(T[:, 0, :], G[:, 1, :], wx_t[:], Alu.mult)
            nc.vector.tensor_tensor(T[:, 1, :], G[:, 2, :], wy_t[:], Alu.mult)
            nc.vector.tensor_tensor(T[:, 2, :], wx_t[:], wy_t[:], Alu.mult)
            nc.vector.tensor_tensor(T[:, 2, :], G[:, 3, :], T[:, 2, :], Alu.mult)
            # (5) accumulate into the deform psum
            for cc in range(4):
                sl = ts(cc, 512)
                for plane, rhs in enumerate(
                    (G[:, 0, sl], T[:, 0, sl], T[:, 1, sl], T[:, 2, sl])
                ):
                    nc.tensor.matmul(
                        dps[:, sl],
                        lhsT=w_def_sb[:, k, :],
                        rhs=rhs,
                        start=(k == 0 and plane == 0),
                        stop=(k == 8 and plane == 3),
                    )
        # ---------------- stage E: aligned*gate + high ----------------
        for cc in range(4):
            gsl = ts(q * 4 + cc, 512)
            t1 = spool.tile([128, 512], F32, tag="as1")
            nc.vector.scalar_tensor_tensor(
                out=t1[:], in0=dps[:, ts(cc, 512)], scalar=b_def_sb[:],
                in1=gate_sb[:, gsl], op0=Alu.add, op1=Alu.mult,
            )
            hc = s1pool.tile([128, 512], F32, tag="hc")
            nc.sync.dma_start(hc[:], io["highc"][:, gsl])
            nc.vector.tensor_tensor(t1[:], t1[:], hc[:], Alu.add)
            nc.sync.dma_start(out_d[:, gsl], t1[:])

    ctx.close()


# ======================= host side =======================

def _prep_shared(w_off, b_off, w_def, b_def, w_mod, b_mod):
    w_off_g = w_off[PERM]                      # [18, 256, 3, 3]
    w_off_t = np.ascontiguousarray(
        w_off_g.reshape(18, 2, 128, 3, 3).transpose(1, 3, 4, 2, 0)
    ).astype(np.float16)                       # [2,3,3,128,18]
    b_off_g = b_off[PERM].reshape(18, 1).astype(np.float32)
    w_def_t = np.ascontiguousarray(
        w_def.reshape(128, 128, 9).transpose(2, 1, 0)
    ).astype(np.float16)                       # [9, c, o]
    w_mod_t = np.ascontiguousarray(
        w_mod.reshape(128, 2, 128).transpose(1, 2, 0)
    ).astype(np.float16)                       # [2, c, o]
    return dict(
        w_off_t=w_off_t,
        b_off_g=b_off_g,
        w_def_t=w_def_t,
        b_def_c=b_def.reshape(128, 1).astype(np.float32),
        w_mod_t=w_mod_t,
        b_mod_c=b_mod.reshape(128, 1).astype(np.float32),
    )


def _prep_src4(low_b):
    """4-plane pixel-major monomial table of the guard-padded low image."""
    xp = np.zeros((C, HP, HP), np.float32)
    xp[:, GP : GP + H, GP : GP + W] = low_b
    f = xp.reshape(C, NP)
    p0 = f
    p1 = np.zeros_like(f)
    p1[:, :-1] = f[:, 1:] - f[:, :-1]
    p2 = np.zeros_like(f)
    p2[:, :-HP] = f[:, HP:] - f[:, :-HP]
    p3 = np.zeros_like(f)
    p3[:, : -HP - 1] = f[:, HP + 1 :] - f[:, HP:-1] - f[:, 1 : -HP] + f[:, : -HP - 1]
    planes = np.stack([p0, p1, p2, p3], 0)      # [4, C, NP]
    return np.ascontiguousarray(planes.transpose(2, 0, 1)).astype(
        np.float16
    ).reshape(NP, 512)


def _prep_base(h0):
    """Sampling-position bases in wrapped order, rows grouped [9 x py, 9 x px]."""
    base = np.empty((18, NPIX), np.float32)
    jj = np.arange(NPIX)
    # wrapped order: within quarter q, column t = p16*128 + s holds the pixel
    # j = s*16 + p16 (local to the quarter)
    q = jj // QP
    tloc = jj % QP
    p16 = tloc // 128
    s = tloc % 128
    jloc = s * 16 + p16
    h = h0 + (q * QP + jloc) // W
    w = (q * QP + jloc) % W
    for k in range(9):
        ky, kx = k // 3, k % 3
        base[k] = h + (ky - 1) + GP
        base[9 + k] = w + (kx - 1) + GP
    return base.astype(np.float16)


def _prep_core(low_b, high_b, h0):
    lp = np.pad(low_b, ((0, 0), (1, 1), (1, 1)))
    hp = np.pad(high_b, ((0, 0), (1, 1), (1, 1)))
    lowp = np.ascontiguousarray(lp[:, h0 : h0 + 66, :]).reshape(128, -1).astype(
        np.float16
    )
    highp = np.ascontiguousarray(hp[:, h0 : h0 + 66, :]).reshape(128, -1).astype(
        np.float16
    )
    highc = np.ascontiguousarray(high_b[:, h0 : h0 + HR, :]).reshape(128, -1).astype(
        np.float32
    )
    return lowp, highp, highc


_PROGRAM_CACHE = {}
_LAST_IN_MAPS = None


def make_in_maps(low_res, high_res, w_off, b_off, w_def, b_def, w_mod, b_mod):
    shared = _prep_shared(
        np.asarray(w_off, np.float32), np.asarray(b_off, np.float32),
        np.asarray(w_def, np.float32), np.asarray(b_def, np.float32),
        np.asarray(w_mod, np.float32), np.asarray(b_mod, np.float32),
    )
    low_res = np.asarray(low_res, np.float32)
    high_res = np.asarray(high_res, np.float32)
    src4_by_batch = [_prep_src4(low_res[b]) for b in range(B)]
    in_maps = []
    for core in range(8):
        b, half = core // 2, core % 2
        h0 = half * HR
        lowp, highp, highc = _prep_core(low_res[b], high_res[b], h0)
        m = dict(shared)
        m["src4"] = src4_by_batch[b]
        m["lowp"] = lowp
        m["highp"] = highp
        m["highc"] = highc
        m["base_w"] = _prep_base(h0)
        in_maps.append(m)
    return in_maps


def kernel(low_res, high_res, w_off, b_off, w_def, b_def, w_mod, b_mod):
    global _LAST_IN_MAPS
    if "nc" not in _PROGRAM_CACHE:
        _PROGRAM_CACHE["nc"] = build_program()
    nc = _PROGRAM_CACHE["nc"]

    in_maps = make_in_maps(
        low_res, high_res, w_off, b_off, w_def, b_def, w_mod, b_mod
    )
    _LAST_IN_MAPS = in_maps

    from concourse import bass_utils

    res = bass_utils.run_bass_kernel_spmd(nc, in_maps, core_ids=list(range(8)))
    out = np.empty((B, C, H, W), np.float32)
    for core in range(8):
        b, half = core // 2, core % 2
        out[b, :, half * HR : half * HR + HR, :] = (
            res.results[core]["out"].reshape(C, HR, W)
        )
    return out

